# revision 1
# baseline (speedup 1.0000x reference)
"""Trainium2 Bass kernel for quadratic non-softmax attention:

    X[n,c,i] = sum_j exp(a[n,c,i] * b[n,c,j]) * v[n,c,j]

with a=fxA, b=fyA, v=fyB, each (2,16,32,32) fp32 -> 32 independent
(n,c) slices of an HWxHW=1024x1024 problem. Sharded 4 slices/core
across 8 NeuronCores (pure data parallel).

Two implementations:

v1 (direct): per i-tile, ScalarE computes E[p,j]=Exp(a[i_p]*b[j]) with a
per-partition `scale`, VectorE's fused affine_mul_reduce does
sum_j E[p,j]*v[j]. ~32 ScalarE exp passes of 1024 elements per core.

v2 (grid-factored, default): quantize a_i = g_k + r_i on a 64-point grid
g_k=(k-32)*h, h=5/32 (exact in fp32, so r is computed exactly). Then
  exp(a_i b_j) = exp(g_k b_j) * exp(r_i b_j)
  X[i] = sum_d r_i^d * H_d[k_i],  H_d[k] = sum_j e^{g_k b_j} v_j b_j^d/d!
(degree-7 Taylor in r*b, |r*b| <= (h/2)*max|b| ~ 0.37 -> trunc err ~8e-9
relative). ScalarE only computes the 128x64 table exp(b_j*g_k) per
j-tile (16x less exp work); H_d comes from one accumulated fp32 matmul
per j-tile; the per-i gather H[k_i,:] is a one-hot fp32 matmul; the
Taylor evaluation is an elementwise multiply with precomputed r^d powers
plus a free-axis reduce_sum on VectorE.
"""

import os

import numpy as np

import concourse.bass as bass
import concourse.mybir as mybir
import concourse.tile as tile
from concourse import bacc
from concourse.bass_utils import run_bass_kernel_spmd

N_CORES = 8
N_SLICES = 32          # N*C = 2*16
S = N_SLICES // N_CORES  # 4 slices per core
HW = 1024              # H*W = 32*32
P = 128                # partitions
NT = HW // P           # 8 tiles of 128
OUT_SHAPE = (2, 16, 32, 32)
F32 = mybir.dt.float32
I32 = mybir.dt.int32

GRID_K = 64            # grid points (= H matmul output partitions)
GRID_H = 5.0 / 32.0    # grid step; exact in fp32 so r = a - k*h is exact
GRID_LO = -5.0         # grid value of index 0; covers |a| <= 5 + h/2
D = 8                  # Taylor terms d=0..7; |r*b| <= (h/2)*4.7 ~ 0.37,
                       # truncation |rb|^8/8! ~ 8e-9 << fp32 noise
BF16 = mybir.dt.bfloat16


def _new_nc():
    return bacc.Bacc(
        "TRN2",
        target_bir_lowering=False,
        debug=False,
        num_devices=N_CORES,
    )


def build_nc_v1() -> bass.Bass:
    nc = _new_nc()
    a = nc.dram_tensor("a", [S, HW], F32, kind="ExternalInput")
    b = nc.dram_tensor("b", [S, HW], F32, kind="ExternalInput")
    v = nc.dram_tensor("v", [S, HW], F32, kind="ExternalInput")
    x = nc.dram_tensor("x", [S, HW], F32, kind="ExternalOutput")

    with tile.TileContext(nc) as tc:
        with (
            tc.tile_pool(name="bcast", bufs=2) as bcast,
            tc.tile_pool(name="small", bufs=2) as small,
            tc.tile_pool(name="work", bufs=4) as work,
        ):
            for s in range(S):
                acol = small.tile([P, NT], F32, tag="acol")
                nc.sync.dma_start(
                    out=acol, in_=a[s].rearrange("(p t) -> p t", p=P)
                )
                bb = bcast.tile([P, HW], F32, tag="bb")
                nc.sync.dma_start(
                    out=bb, in_=b[s : s + 1, :].to_broadcast((P, HW))
                )
                vb = bcast.tile([P, HW], F32, tag="vb")
                nc.sync.dma_start(
                    out=vb, in_=v[s : s + 1, :].to_broadcast((P, HW))
                )
                xcol = small.tile([P, NT], F32, tag="xcol")
                for t in range(NT):
                    e = work.tile([P, HW], F32, tag="e")
                    nc.scalar.activation(
                        out=e,
                        in_=bb,
                        func=mybir.ActivationFunctionType.Exp,
                        scale=acol[:, t : t + 1],
                    )
                    # prod = e * vb ; xcol[:, t] = sum_j prod[:, j]
                    prod = work.tile([P, HW], F32, tag="prod")
                    nc.vector.affine_mul_reduce(
                        out=prod,
                        accum_out=xcol[:, t : t + 1],
                        in0=e,
                        in1=vb,
                        scale=1.0,
                        bias=0.0,
                    )
                nc.sync.dma_start(
                    out=x[s].rearrange("(p t) -> p t", p=P), in_=xcol
                )
    nc.compile()
    return nc


def build_nc_v2() -> bass.Bass:
    nc = _new_nc()
    a = nc.dram_tensor("a", [S, HW], F32, kind="ExternalInput")
    b = nc.dram_tensor("b", [S, HW], F32, kind="ExternalInput")
    v = nc.dram_tensor("v", [S, HW], F32, kind="ExternalInput")
    x = nc.dram_tensor("x", [S, HW], F32, kind="ExternalOutput")

    # index mapping: i = p*NT + t (and j = p*NT + u) so the DRAM side of
    # every strided DMA moves NT contiguous elements per (partition, slice)
    col_view = lambda t: t.rearrange("s (p t) -> p s t", p=P)

    with tile.TileContext(nc) as tc:
        with (
            tc.tile_pool(name="const", bufs=1) as const,
            tc.tile_pool(name="cols", bufs=1) as cols,
            tc.tile_pool(name="gt", bufs=8) as gtp,
            tc.tile_pool(name="oh", bufs=1) as ohp,
            tc.tile_pool(name="hsb", bufs=2) as hsbp,
            tc.tile_pool(name="hps", bufs=2, space="PSUM") as hps,
            tc.tile_pool(name="coefps", bufs=1, space="PSUM") as coefps,
            tc.tile_pool(name="dram", bufs=1, space="DRAM") as dram,
        ):
            # ---- constants ----
            io32 = const.tile([P, GRID_K], I32, tag="io32")
            nc.gpsimd.iota(
                out=io32, pattern=[[1, GRID_K]], base=0, channel_multiplier=0
            )
            iof = const.tile([P, GRID_K], F32, tag="iof")
            nc.vector.tensor_copy(out=iof, in_=io32)
            # gridb[p, k] = k*h + GRID_LO, same on every partition (exact fp32)
            gridb = const.tile([P, GRID_K], F32, tag="gridb")
            nc.scalar.activation(
                out=gridb,
                in_=iof,
                func=mybir.ActivationFunctionType.Copy,
                scale=GRID_H,
                bias=GRID_LO,
            )
            pc32 = const.tile([GRID_K, 1], I32, tag="pc32")
            nc.gpsimd.iota(
                out=pc32,
                pattern=[[0, 1]],
                base=int(GRID_LO / GRID_H),
                channel_multiplier=1,
            )
            pcf = const.tile([GRID_K, 1], F32, tag="pcf")
            nc.vector.tensor_copy(out=pcf, in_=pc32)

            # ---- batched input loads (column layout: [p, s, t]) ----
            acol = cols.tile([P, S, NT], F32, tag="acol")
            nc.sync.dma_start(out=acol, in_=col_view(a))
            bcol = cols.tile([P, S, NT], F32, tag="bcol")
            nc.sync.dma_start(out=bcol, in_=col_view(b))
            vcol = cols.tile([P, S, NT], F32, tag="vcol")
            nc.sync.dma_start(out=vcol, in_=col_view(v))

            # ---- grid index + remainder (column layout, all slices) ----
            # a*(1/h) with int32 output dtype = round-to-nearest in one op
            ki32 = cols.tile([P, S, NT], I32, tag="ki32")
            nc.vector.tensor_scalar_mul(out=ki32, in0=acol, scalar1=1.0 / GRID_H)
            kf = cols.tile([P, S, NT], BF16, tag="kf")
            nc.vector.tensor_copy(out=kf, in_=ki32)
            # r = a - kf*h  (exact: kf*h is exact for h=5/32, |kf|<=32)
            rc = cols.tile([P, S, NT], F32, tag="rc")
            nc.vector.scalar_tensor_tensor(
                out=rc, in0=kf, scalar=-GRID_H, in1=acol,
                op0=mybir.AluOpType.mult, op1=mybir.AluOpType.add,
            )

            # roundtrip through DRAM to get row layout for the one-hot
            # compare; kf holds k' = k-32 (exact small ints in bf16), so the
            # partition-index constant below is built with base -32
            kinl = dram.tile([S, HW], BF16, tag="kinl")
            nc.sync.dma_start(out=col_view(kinl), in_=kf)

            # ---- Taylor weights W[p, s, t, d] = v * b^d / d! ----
            W = cols.tile([P, S, NT, D], F32, tag="W")
            nc.vector.tensor_copy(out=W[:, :, :, 0], in_=vcol)
            for d in range(1, D):
                # W_d = (W_{d-1} * (1/d)) * b
                nc.vector.scalar_tensor_tensor(
                    out=W[:, :, :, d],
                    in0=W[:, :, :, d - 1],
                    scalar=1.0 / d,
                    in1=bcol,
                    op0=mybir.AluOpType.mult,
                    op1=mybir.AluOpType.mult,
                )

            # ---- powers RD[p, s, t, d] = r^d (early; off the critical path) ----
            RD = cols.tile([P, S, NT, D], F32, tag="RD")
            nc.vector.memset(RD[:, :, :, 0], 1.0)
            for d in range(1, D):
                nc.vector.tensor_mul(
                    out=RD[:, :, :, d], in0=RD[:, :, :, d - 1], in1=rc
                )

            coef = coefps.tile([P, S, NT, D], F32, tag="coef")

            # ---- one-hot rows for every slice, computed up front so the
            # gather matmuls never wait on the kib broadcast DMAs ----
            ohs = []
            for pair in range(S // 2):
                # one broadcast DMA per slice-pair (halves the per-DMA fixed
                # cost paid on this chain); is_equal stays per slice
                kib2 = ohp.tile([GRID_K, 2, HW], BF16, tag=f"kib{pair}")
                kv = kinl[2 * pair : 2 * pair + 2, :]
                nc.sync.dma_start(
                    out=kib2,
                    in_=bass.AP(
                        kv.tensor,
                        kv.offset,
                        [[0, GRID_K]] + [list(d) for d in kv.ap],
                    ),
                )
                for j in range(2):
                    oh = ohp.tile([GRID_K, HW], F32, tag=f"oh{2 * pair + j}")
                    nc.vector.tensor_scalar(
                        out=oh,
                        in0=kib2[:, j, :],
                        scalar1=pcf,
                        scalar2=None,
                        op0=mybir.AluOpType.is_equal,
                    )
                    ohs.append(oh)

            for s in range(S):
                # ---- H_d[k] = sum_j e^{g_k b_j} v_j b_j^d/d!  (PSUM accum) ----
                # arg[p, u, k] = b[p,s,u] * g_k via broadcast APs (one DVE op),
                # then ONE big exp per slice (the per-instruction ACT overhead
                # dominates small tiles, so batch all 8 j-tiles' tables)
                arg = gtp.tile([P, NT, GRID_K], F32, tag="arg")
                bs = bcol[:, s, :]
                b_b = bass.AP(
                    bs.tensor, bs.offset,
                    [list(d) for d in bs.ap] + [[0, GRID_K]],
                )
                g_b = bass.AP(
                    gridb.tensor, gridb.offset,
                    [list(gridb.ap[0]), [0, NT], list(gridb.ap[1])],
                )
                nc.vector.tensor_tensor(
                    out=arg, in0=b_b, in1=g_b, op=mybir.AluOpType.mult
                )
                gt = gtp.tile([P, NT, GRID_K], F32, tag="gt")
                nc.scalar.activation(
                    out=gt, in_=arg, func=mybir.ActivationFunctionType.Exp
                )
                Hps = hps.tile([GRID_K, D], F32, tag="H")
                for u in range(NT):
                    nc.tensor.matmul(
                        out=Hps,
                        lhsT=gt[:, u, :],
                        rhs=W[:, s, u, :],
                        start=(u == 0),
                        stop=(u == NT - 1),
                    )
                Hsb = hsbp.tile([GRID_K, D], F32, tag="Hsb")
                # PSUM->SBUF handoff on the (idle) scalar engine so it never
                # queues behind VectorE work and PSUM banks recycle promptly
                nc.scalar.copy(out=Hsb, in_=Hps)

                # ---- one-hot gather: coef[i, :] = H[k_i, :] ----
                # free position in oh is i = c*NT + u; tile u gathers the
                # i's congruent to u (strided columns), matching coef[c,s,u]
                oh_r = ohs[s].rearrange("p (c t) -> p t c", t=NT)
                for u in range(NT):
                    nc.tensor.matmul(
                        out=coef[:, s, u, :],
                        lhsT=oh_r[:, u, :],
                        rhs=Hsb,
                        start=True,
                        stop=True,
                    )

            # ---- X = sum_d r^d * H_d[k_i]: multiply + reduce, in two halves
            # so the first half's store overlaps the second half's compute ----
            half = S // 2
            for lo in (0, half):
                hi = lo + half
                prodc = cols.tile([P, half, NT, D], F32, tag=f"prodc{lo}")
                nc.vector.tensor_mul(
                    out=prodc, in0=RD[:, lo:hi, :, :], in1=coef[:, lo:hi, :, :]
                )
                xall = cols.tile([P, half, NT], F32, tag=f"xall{lo}")
                nc.vector.reduce_sum(
                    out=xall, in_=prodc, axis=mybir.AxisListType.X
                )
                nc.sync.dma_start(
                    out=x[lo:hi].rearrange("s (p t) -> p s t", p=P), in_=xall
                )
    nc.compile()
    return nc


_NC_CACHE = {}
_VERSION = os.environ.get("KERNEL_VERSION", "v2")


def _get_nc():
    ver = _VERSION
    if ver not in _NC_CACHE:
        _NC_CACHE[ver] = build_nc_v2() if ver == "v2" else build_nc_v1()
    return _NC_CACHE[ver]


def kernel(fxA, fyA, fyB, _trace=False, _tmpdir=None):
    a_full = np.ascontiguousarray(np.asarray(fxA), dtype=np.float32).reshape(
        N_SLICES, HW
    )
    b_full = np.ascontiguousarray(np.asarray(fyA), dtype=np.float32).reshape(
        N_SLICES, HW
    )
    v_full = np.ascontiguousarray(np.asarray(fyB), dtype=np.float32).reshape(
        N_SLICES, HW
    )

    in_maps = []
    for c in range(N_CORES):
        lo, hi = c * S, (c + 1) * S
        in_maps.append(
            {"a": a_full[lo:hi], "b": b_full[lo:hi], "v": v_full[lo:hi]}
        )

    res = run_bass_kernel_spmd(
        _get_nc(),
        in_maps,
        core_ids=list(range(N_CORES)),
        trace=_trace,
        tmpdir=_tmpdir,
    )
    out = np.concatenate([r["x"] for r in res.results], axis=0)
    if _trace:
        kernel.last_results = res
    return out.reshape(OUT_SHAPE).astype(np.float32)



# revision 43
# speedup vs baseline: 1.1688x; 1.1688x over previous
"""Trainium2 Bass kernel for quadratic non-softmax attention:

    X[n,c,i] = sum_j exp(a[n,c,i] * b[n,c,j]) * v[n,c,j]

with a=fxA, b=fyA, v=fyB, each (2,16,32,32) fp32 -> 32 independent
(n,c) slices of an HWxHW=1024x1024 problem. Sharded 4 slices/core
across 8 NeuronCores (pure data parallel).

Two implementations:

v1 (direct): per i-tile, ScalarE computes E[p,j]=Exp(a[i_p]*b[j]) with a
per-partition `scale`, VectorE's fused affine_mul_reduce does
sum_j E[p,j]*v[j]. ~32 ScalarE exp passes of 1024 elements per core.

v2 (grid-factored, default): quantize a_i = g_k + r_i on a 64-point grid
g_k=(k-32)*h, h=5/32 (exact in fp32, so r is computed exactly). Then
  exp(a_i b_j) = exp(g_k b_j) * exp(r_i b_j)
  X[i] = sum_d r_i^d * H_d[k_i],  H_d[k] = sum_j e^{g_k b_j} v_j b_j^d/d!
(degree-7 Taylor in r*b, |r*b| <= (h/2)*max|b| ~ 0.37 -> trunc err ~8e-9
relative). ScalarE only computes the 128x64 table exp(b_j*g_k) per
j-tile (16x less exp work); H_d comes from one accumulated fp32 matmul
per j-tile; the per-i gather H[k_i,:] is a one-hot fp32 matmul; the
Taylor evaluation is an elementwise multiply with precomputed r^d powers
plus a free-axis reduce_sum on VectorE.
"""

import os

import numpy as np

import concourse.bass as bass
import concourse.mybir as mybir
import concourse.tile as tile
from concourse import bacc
from concourse.bass_utils import run_bass_kernel_spmd

N_CORES = 8
N_SLICES = 32          # N*C = 2*16
S = N_SLICES // N_CORES  # 4 slices per core
HW = 1024              # H*W = 32*32
P = 128                # partitions
NT = HW // P           # 8 tiles of 128
OUT_SHAPE = (2, 16, 32, 32)
F32 = mybir.dt.float32
I32 = mybir.dt.int32

GRID_K = 64            # grid points (= H matmul output partitions)
GRID_H = 5.0 / 32.0    # grid step; exact in fp32 so r = a - k*h is exact
GRID_LO = -5.0         # grid value of index 0; covers |a| <= 5 + h/2
D = 8                  # Taylor terms d=0..7; |r*b| <= (h/2)*4.7 ~ 0.37,
                       # truncation |rb|^8/8! ~ 8e-9 << fp32 noise
BF16 = mybir.dt.bfloat16


def _new_nc():
    return bacc.Bacc(
        "TRN2",
        target_bir_lowering=False,
        debug=False,
        num_devices=N_CORES,
    )


def build_nc_v1() -> bass.Bass:
    nc = _new_nc()
    a = nc.dram_tensor("a", [S, HW], F32, kind="ExternalInput")
    b = nc.dram_tensor("b", [S, HW], F32, kind="ExternalInput")
    v = nc.dram_tensor("v", [S, HW], F32, kind="ExternalInput")
    x = nc.dram_tensor("x", [S, HW], F32, kind="ExternalOutput")

    with tile.TileContext(nc) as tc:
        with (
            tc.tile_pool(name="bcast", bufs=2) as bcast,
            tc.tile_pool(name="small", bufs=2) as small,
            tc.tile_pool(name="work", bufs=4) as work,
        ):
            for s in range(S):
                acol = small.tile([P, NT], F32, tag="acol")
                nc.sync.dma_start(
                    out=acol, in_=a[s].rearrange("(p t) -> p t", p=P)
                )
                bb = bcast.tile([P, HW], F32, tag="bb")
                nc.sync.dma_start(
                    out=bb, in_=b[s : s + 1, :].to_broadcast((P, HW))
                )
                vb = bcast.tile([P, HW], F32, tag="vb")
                nc.sync.dma_start(
                    out=vb, in_=v[s : s + 1, :].to_broadcast((P, HW))
                )
                xcol = small.tile([P, NT], F32, tag="xcol")
                for t in range(NT):
                    e = work.tile([P, HW], F32, tag="e")
                    nc.scalar.activation(
                        out=e,
                        in_=bb,
                        func=mybir.ActivationFunctionType.Exp,
                        scale=acol[:, t : t + 1],
                    )
                    # prod = e * vb ; xcol[:, t] = sum_j prod[:, j]
                    prod = work.tile([P, HW], F32, tag="prod")
                    nc.vector.affine_mul_reduce(
                        out=prod,
                        accum_out=xcol[:, t : t + 1],
                        in0=e,
                        in1=vb,
                        scale=1.0,
                        bias=0.0,
                    )
                nc.sync.dma_start(
                    out=x[s].rearrange("(p t) -> p t", p=P), in_=xcol
                )
    nc.compile()
    return nc


def build_nc_v2() -> bass.Bass:
    nc = _new_nc()
    a = nc.dram_tensor("a", [S, HW], F32, kind="ExternalInput")
    b = nc.dram_tensor("b", [S, HW], F32, kind="ExternalInput")
    v = nc.dram_tensor("v", [S, HW], F32, kind="ExternalInput")
    x = nc.dram_tensor("x", [S, HW], F32, kind="ExternalOutput")

    # index mapping: i = p*NT + t (and j = p*NT + u) so the DRAM side of
    # every strided DMA moves NT contiguous elements per (partition, slice)
    col_view = lambda t: t.rearrange("s (p t) -> p s t", p=P)

    with tile.TileContext(nc) as tc:
        with (
            tc.tile_pool(name="const", bufs=1) as const,
            tc.tile_pool(name="cols", bufs=1) as cols,
            tc.tile_pool(name="gt", bufs=8) as gtp,
            tc.tile_pool(name="oh", bufs=1) as ohp,
            tc.tile_pool(name="hsb", bufs=2) as hsbp,
            tc.tile_pool(name="hps", bufs=2, space="PSUM") as hps,
            tc.tile_pool(name="coefps", bufs=1, space="PSUM") as coefps,
            tc.tile_pool(name="dram", bufs=1, space="DRAM") as dram,
        ):
            # ---- constants ----
            io32 = const.tile([P, GRID_K], I32, tag="io32")
            nc.gpsimd.iota(
                out=io32, pattern=[[1, GRID_K]], base=0, channel_multiplier=0
            )
            iof = const.tile([P, GRID_K], F32, tag="iof")
            nc.vector.tensor_copy(out=iof, in_=io32)
            # gridb[p, k] = k*h + GRID_LO, same on every partition (exact fp32)
            gridb = const.tile([P, GRID_K], F32, tag="gridb")
            nc.scalar.activation(
                out=gridb,
                in_=iof,
                func=mybir.ActivationFunctionType.Copy,
                scale=GRID_H,
                bias=GRID_LO,
            )
            pc32 = const.tile([GRID_K, 1], I32, tag="pc32")
            nc.gpsimd.iota(
                out=pc32,
                pattern=[[0, 1]],
                base=int(GRID_LO / GRID_H),
                channel_multiplier=1,
            )
            pcf = const.tile([GRID_K, 1], F32, tag="pcf")
            nc.vector.tensor_copy(out=pcf, in_=pc32)

            # ---- batched input loads (column layout: [p, s, t]) ----
            acol = cols.tile([P, S, NT], F32, tag="acol")
            nc.sync.dma_start(out=acol, in_=col_view(a))
            bcol = cols.tile([P, S, NT], F32, tag="bcol")
            nc.sync.dma_start(out=bcol, in_=col_view(b))
            vcol = cols.tile([P, S, NT], F32, tag="vcol")
            nc.sync.dma_start(out=vcol, in_=col_view(v))

            # ---- grid index + remainder (column layout, all slices) ----
            # a*(1/h) with int32 output dtype = round-to-nearest in one op
            ki32 = cols.tile([P, S, NT], I32, tag="ki32")
            nc.vector.tensor_scalar_mul(out=ki32, in0=acol, scalar1=1.0 / GRID_H)
            kf = cols.tile([P, S, NT], BF16, tag="kf")
            nc.vector.tensor_copy(out=kf, in_=ki32)
            # r = a - kf*h  (exact: kf*h is exact for h=5/32, |kf|<=32)
            rc = cols.tile([P, S, NT], F32, tag="rc")
            nc.vector.scalar_tensor_tensor(
                out=rc, in0=kf, scalar=-GRID_H, in1=acol,
                op0=mybir.AluOpType.mult, op1=mybir.AluOpType.add,
            )

            # roundtrip through DRAM to get row layout for the one-hot
            # compare; kf holds k' = k-32 (exact small ints in bf16), so the
            # partition-index constant below is built with base -32
            kinl = dram.tile([S, HW], BF16, tag="kinl")
            nc.sync.dma_start(out=col_view(kinl), in_=kf)

            # ---- Taylor weights W[p, s, t, d] = v * b^d / d! ----
            W = cols.tile([P, S, NT, D], F32, tag="W")
            nc.vector.tensor_copy(out=W[:, :, :, 0], in_=vcol)
            for d in range(1, D):
                # W_d = (W_{d-1} * (1/d)) * b
                nc.vector.scalar_tensor_tensor(
                    out=W[:, :, :, d],
                    in0=W[:, :, :, d - 1],
                    scalar=1.0 / d,
                    in1=bcol,
                    op0=mybir.AluOpType.mult,
                    op1=mybir.AluOpType.mult,
                )

            # ---- powers RD[p, s, t, d] = r^d (early; off the critical path) ----
            RD = cols.tile([P, S, NT, D], F32, tag="RD")
            nc.vector.memset(RD[:, :, :, 0], 1.0)
            for d in range(1, D):
                nc.vector.tensor_mul(
                    out=RD[:, :, :, d], in0=RD[:, :, :, d - 1], in1=rc
                )

            coef = coefps.tile([P, S, NT, D], F32, tag="coef")

            # ---- one-hot rows for every slice, computed up front so the
            # gather matmuls never wait on the kib broadcast DMAs ----
            ohs = []
            for pair in range(S // 2):
                # one broadcast DMA per slice-pair (halves the per-DMA fixed
                # cost paid on this chain); is_equal stays per slice
                kib2 = ohp.tile([GRID_K, 2, HW], BF16, tag=f"kib{pair}")
                kv = kinl[2 * pair : 2 * pair + 2, :]
                nc.sync.dma_start(
                    out=kib2,
                    in_=bass.AP(
                        kv.tensor,
                        kv.offset,
                        [[0, GRID_K]] + [list(d) for d in kv.ap],
                    ),
                )
                for j in range(2):
                    oh = ohp.tile([GRID_K, HW], F32, tag=f"oh{2 * pair + j}")
                    nc.vector.tensor_scalar(
                        out=oh,
                        in0=kib2[:, j, :],
                        scalar1=pcf,
                        scalar2=None,
                        op0=mybir.AluOpType.is_equal,
                    )
                    ohs.append(oh)

            for s in range(S):
                # ---- H_d[k] = sum_j e^{g_k b_j} v_j b_j^d/d!  (PSUM accum) ----
                # arg[p, u, k] = b[p,s,u] * g_k via broadcast APs (one DVE op),
                # then ONE big exp per slice (the per-instruction ACT overhead
                # dominates small tiles, so batch all 8 j-tiles' tables)
                arg = gtp.tile([P, NT, GRID_K], F32, tag="arg")
                bs = bcol[:, s, :]
                b_b = bass.AP(
                    bs.tensor, bs.offset,
                    [list(d) for d in bs.ap] + [[0, GRID_K]],
                )
                g_b = bass.AP(
                    gridb.tensor, gridb.offset,
                    [list(gridb.ap[0]), [0, NT], list(gridb.ap[1])],
                )
                nc.vector.tensor_tensor(
                    out=arg, in0=b_b, in1=g_b, op=mybir.AluOpType.mult
                )
                gt = gtp.tile([P, NT, GRID_K], F32, tag="gt")
                nc.scalar.activation(
                    out=gt, in_=arg, func=mybir.ActivationFunctionType.Exp
                )
                Hps = hps.tile([GRID_K, D], F32, tag="H")
                for u in range(NT):
                    nc.tensor.matmul(
                        out=Hps,
                        lhsT=gt[:, u, :],
                        rhs=W[:, s, u, :],
                        start=(u == 0),
                        stop=(u == NT - 1),
                    )
                Hsb = hsbp.tile([GRID_K, D], F32, tag="Hsb")
                # PSUM->SBUF handoff on the (idle) scalar engine so it never
                # queues behind VectorE work and PSUM banks recycle promptly
                nc.scalar.copy(out=Hsb, in_=Hps)

                # ---- one-hot gather: coef[i, :] = H[k_i, :] ----
                # free position in oh is i = c*NT + u; tile u gathers the
                # i's congruent to u (strided columns), matching coef[c,s,u]
                oh_r = ohs[s].rearrange("p (c t) -> p t c", t=NT)
                for u in range(NT):
                    nc.tensor.matmul(
                        out=coef[:, s, u, :],
                        lhsT=oh_r[:, u, :],
                        rhs=Hsb,
                        start=True,
                        stop=True,
                    )

            # ---- X = sum_d r^d * H_d[k_i]: multiply + reduce, in two halves
            # so the first half's store overlaps the second half's compute ----
            half = S // 2
            for lo in (0, half):
                hi = lo + half
                prodc = cols.tile([P, half, NT, D], F32, tag=f"prodc{lo}")
                nc.vector.tensor_mul(
                    out=prodc, in0=RD[:, lo:hi, :, :], in1=coef[:, lo:hi, :, :]
                )
                xall = cols.tile([P, half, NT], F32, tag=f"xall{lo}")
                nc.vector.reduce_sum(
                    out=xall, in_=prodc, axis=mybir.AxisListType.X
                )
                nc.sync.dma_start(
                    out=x[lo:hi].rearrange("s (p t) -> p s t", p=P), in_=xall
                )
    nc.compile()
    return nc


K3 = 32                # v3 grid points; h exact in fp32
H3 = 5.0 / 16.0        # 0.3125 = 5 * 2^-4
KOFF = 16              # partition (s*K3 + k) holds grid index k' = k - KOFF
INV_H3 = 1.0 / H3      # fl(3.2); both rounding ops use this same constant


def build_nc_v3() -> bass.Bass:
    """v3: no DRAM roundtrip for the one-hot. `a` is ALSO loaded in a
    row-broadcast layout abc[s*32+k, i] = a[s, i]; GPSIMD rounds both the
    column copy (acol -> ki32, for the Taylor remainder r) and the row
    copy (abc -> kint, for the one-hot compare) with the same op on the
    same engine, so the two roundings are bit-identical by construction.
    DMAs are split across both HWDGE queues (SP + Activation). K=32
    halves the exp-table/arg work vs v2's K=64 (|r*b| <= 0.16*4.4 ->
    Taylor-8 truncation ~1e-6 relative, well below fp32 noise)."""
    nc = _new_nc()
    a = nc.dram_tensor("a", [S, HW], F32, kind="ExternalInput")
    b = nc.dram_tensor("b", [S, HW], F32, kind="ExternalInput")
    v = nc.dram_tensor("v", [S, HW], F32, kind="ExternalInput")
    x = nc.dram_tensor("x", [S, HW], F32, kind="ExternalOutput")

    col_view = lambda t: t.rearrange("s (p t) -> p s t", p=P)

    with tile.TileContext(nc) as tc:
        with (
            tc.tile_pool(name="const", bufs=1) as const,
            tc.tile_pool(name="cols", bufs=1) as cols,
            tc.tile_pool(name="big", bufs=1) as big,
            tc.tile_pool(name="gtp", bufs=2) as gtp,
            tc.tile_pool(name="hsb", bufs=2) as hsbp,
            tc.tile_pool(name="outp", bufs=2) as outp,
            tc.tile_pool(name="hps", bufs=2, space="PSUM") as hps,
            tc.tile_pool(name="coefps", bufs=1, space="PSUM") as coefps,
        ):
            # ---- input DMAs ----
            # TimelineSim serializes HWDGE slots (~625ns each), so ORDER is
            # the schedule: abc gates the longest chain (kint -> oh ->
            # gather), bcol the args -> exp -> H chain.
            nch = int(os.environ.get("V3_ABC_CHUNKS", "2"))
            order = os.environ.get("V3_DMA_ORDER", "b,g0,a,g1,v")
            QW = HW // nch
            abc = big.tile([P, HW], F32, tag="abc")
            aap = a[:, :]
            cols_tiles = {}
            engs = [nc.sync, nc.scalar]
            ei = 0

            def emit_dma(key):
                nonlocal ei
                eng = engs[ei % 2]
                ei += 1
                if key == "b":
                    t = cols.tile([P, S, NT], F32, tag="bcol")
                    eng.dma_start(out=t, in_=col_view(b))
                    cols_tiles["b"] = t
                elif key == "a":
                    t = cols.tile([P, S, NT], F32, tag="acol")
                    eng.dma_start(out=t, in_=col_view(a))
                    cols_tiles["a"] = t
                elif key == "v":
                    t = cols.tile([P, S, NT], F32, tag="vcol")
                    eng.dma_start(out=t, in_=col_view(v))
                    cols_tiles["v"] = t
                elif key.startswith("g"):
                    q = int(key[1:])
                    eng.dma_start(
                        out=abc[:, q * QW : (q + 1) * QW],
                        in_=bass.AP(
                            aap.tensor,
                            aap.offset + q * QW,
                            [[HW, S], [0, K3], [1, QW]],
                        ),
                    )

            for key in order.split(","):
                emit_dma(key)
            bcol = cols_tiles["b"]
            acol = cols_tiles["a"]
            vcol = cols_tiles["v"]

            # ---- constants ----
            io32 = const.tile([P, K3], I32, tag="io32")
            nc.gpsimd.iota(
                out=io32, pattern=[[1, K3]], base=0, channel_multiplier=0
            )
            iof = const.tile([P, K3], F32, tag="iof")
            nc.vector.tensor_copy(out=iof, in_=io32)
            # gridb[p, k] = (k - KOFF) * h, same on every partition
            gridb = const.tile([P, K3], F32, tag="gridb")
            nc.scalar.activation(
                out=gridb,
                in_=iof,
                func=mybir.ActivationFunctionType.Copy,
                scale=H3,
                bias=-KOFF * H3,
            )
            # pci[p] = (p & 31) - KOFF : the k' each oh partition matches
            piota = const.tile([P, 1], I32, tag="piota")
            nc.gpsimd.iota(
                out=piota, pattern=[[0, 1]], base=0, channel_multiplier=1
            )
            pcim = const.tile([P, 1], I32, tag="pcim")
            nc.vector.tensor_scalar(
                out=pcim,
                in0=piota,
                scalar1=K3 - 1,
                scalar2=None,
                op0=mybir.AluOpType.bitwise_and,
            )
            pcii = const.tile([P, 1], I32, tag="pcii")
            nc.vector.tensor_scalar_add(out=pcii, in0=pcim, scalar1=-KOFF)
            pci = const.tile([P, 1], F32, tag="pci")
            nc.vector.tensor_copy(out=pci, in_=pcii)

            # ---- GPSIMD rounds both layouts (bit-identical) ----
            ki32 = cols.tile([P, S, NT], I32, tag="ki32")
            nc.gpsimd.tensor_scalar_mul(out=ki32, in0=acol, scalar1=INV_H3)
            kint = big.tile([P, HW], I32, tag="kint")
            for q in range(nch):
                nc.gpsimd.tensor_scalar_mul(
                    out=kint[:, q * QW : (q + 1) * QW],
                    in0=abc[:, q * QW : (q + 1) * QW],
                    scalar1=INV_H3,
                )

            # ---- DVE: args (pairwise), remainder, powers ----
            arg = big.tile([P, S, NT, K3], F32, tag="arg")
            W = cols.tile([P, S, NT, D], F32, tag="W")
            RD = cols.tile([P, S, NT, D], F32, tag="RD")
            oh = big.tile([P, HW], F32, tag="oh")
            kf = cols.tile([P, S, NT], BF16, tag="kf")
            rc = cols.tile([P, S, NT], F32, tag="rc")

            def arg_pair(lo):
                bs = bcol[:, lo : lo + 2, :]
                b_b = bass.AP(
                    bs.tensor,
                    bs.offset,
                    [list(d) for d in bs.ap] + [[0, K3]],
                )
                g_b = bass.AP(
                    gridb.tensor,
                    gridb.offset,
                    [list(gridb.ap[0]), [0, 2], [0, NT], list(gridb.ap[1])],
                )
                nc.vector.tensor_tensor(
                    out=arg[:, lo : lo + 2], in0=b_b, in1=g_b,
                    op=mybir.AluOpType.mult,
                )

            with tc.high_priority():
                arg_pair(0)
                arg_pair(2)

            # W_d = v * b^d / d! — build b^d/d! on GPSIMD (needs only bcol;
            # keeps DVE free for args), then per-pair multiplies by v on DVE
            B = cols.tile([P, S, NT, D], F32, tag="B")
            nc.vector.memset(B[:, :, :, 0], 1.0)
            for d in range(1, D):
                nc.vector.scalar_tensor_tensor(
                    out=B[:, :, :, d], in0=B[:, :, :, d - 1],
                    scalar=1.0 / d, in1=bcol,
                    op0=mybir.AluOpType.mult, op1=mybir.AluOpType.mult,
                )
            nc.vector.tensor_copy(out=kf, in_=ki32)
            nc.vector.scalar_tensor_tensor(
                out=rc, in0=kf, scalar=-H3, in1=acol,
                op0=mybir.AluOpType.mult, op1=mybir.AluOpType.add,
            )
            for lo in (0, 2):
                vs = vcol[:, lo : lo + 2, :]
                v_b = bass.AP(
                    vs.tensor, vs.offset,
                    [list(dd) for dd in vs.ap] + [[0, D]],
                )
                nc.gpsimd.tensor_tensor(
                    out=W[:, lo : lo + 2], in0=B[:, lo : lo + 2], in1=v_b,
                    op=mybir.AluOpType.mult,
                )
            # one-hot in free-chunks (pipelines behind the kint chunks)
            for q in range(nch):
                nc.vector.tensor_scalar(
                    out=oh[:, q * QW : (q + 1) * QW],
                    in0=kint[:, q * QW : (q + 1) * QW],
                    scalar1=pci,
                    scalar2=None,
                    op0=mybir.AluOpType.is_equal,
                )
            # RD_d = r^d for the final Taylor eval (fills the DVE gap
            # before the kint chunks land)
            nc.vector.memset(RD[:, :, :, 0], 1.0)
            for d in range(1, D):
                nc.vector.tensor_mul(
                    out=RD[:, :, :, d], in0=RD[:, :, :, d - 1], in1=rc
                )

            # ---- ACT: exp tables (pairwise); PE: H matmuls ----
            # Hall[32s + k, d] = H_d^{(s)}[k]: slice s's table lives at
            # partition base 32s (PE tile_position writes PSUM there), so
            # the gather's lhsT (oh rows) and rhs (Hsbd rows) share a base.
            # Hsbd is BLOCK-DIAGONAL per pair ([H_2p | 0; 0 | H_2p+1]) so
            # one matmul gathers both slices of a pair: 16 gathers, not 32.
            coefd0 = coefps.tile([P, NT, 2, D], F32, tag="coefd0")
            coefd1 = coefps.tile([P, NT, 2, D], F32, tag="coefd1")
            coefd = [coefd0, coefd1]
            Hall = hps.tile([P, D], F32, tag="Hall")
            Hsbd = hsbp.tile([P, 2, D], F32, tag="Hsbd")
            nc.vector.memset(Hsbd, 0.0)
            for pair in range(2):
                lo = 2 * pair
                gt = gtp.tile([P, 2, NT, K3], F32, tag=f"gt{pair}")
                nc.scalar.activation(
                    out=gt, in_=arg[:, lo : lo + 2],
                    func=mybir.ActivationFunctionType.Exp,
                )
                for j in range(2):
                    s = lo + j
                    for u in range(NT):
                        nc.tensor.matmul(
                            out=Hall[K3 * s : K3 * s + K3, :],
                            lhsT=gt[:, j, u, :],
                            rhs=W[:, s, u, :],
                            start=(u == 0),
                            stop=(u == NT - 1),
                            tile_position=(0, K3 * s),
                        )
                    nc.scalar.copy(
                        out=Hsbd[K3 * s : K3 * s + K3, j, :],
                        in_=Hall[K3 * s : K3 * s + K3, :],
                    )
                # merged gather: coefd[c, pair, u, j, :] = H_{2p+j}[k'_..., :]
                oh_r = oh[64 * pair : 64 * pair + 64].rearrange(
                    "k (c t) -> k t c", t=NT
                )
                for u in range(NT):
                    nc.tensor.matmul(
                        out=coefd[pair][:, u, :, :],
                        lhsT=oh_r[:, u, :],
                        rhs=Hsbd[64 * pair : 64 * pair + 64, :, :],
                        start=True,
                        stop=True,
                        tile_position=(64 * pair, 0),
                    )

            # ---- final eval + stores, pairwise so store0 overlaps pair 1 ----
            prodc = outp.tile([P, S, NT, D], F32, tag="prodc")
            xall = outp.tile([P, S, NT], F32, tag="xall")
            for pair, eng in ((0, nc.sync), (1, nc.scalar)):
                lo = 2 * pair
                nc.vector.tensor_mul(
                    out=prodc[:, lo : lo + 2],
                    in0=RD[:, lo : lo + 2],
                    in1=coefd[pair].rearrange("p t j d -> p j t d"),
                )
                nc.vector.reduce_sum(
                    out=xall[:, lo : lo + 2],
                    in_=prodc[:, lo : lo + 2],
                    axis=mybir.AxisListType.X,
                )
                eng.dma_start(
                    out=x[lo : lo + 2].rearrange("s (p t) -> p s t", p=P),
                    in_=xall[:, lo : lo + 2],
                )
    nc.compile()
    return nc


_NC_CACHE = {}
_VERSION = os.environ.get("KERNEL_VERSION", "v3")


def _get_nc():
    ver = _VERSION
    if ver not in _NC_CACHE:
        builders = {"v1": build_nc_v1, "v2": build_nc_v2, "v3": build_nc_v3}
        _NC_CACHE[ver] = builders[ver]()
    return _NC_CACHE[ver]


def kernel(fxA, fyA, fyB, _trace=False, _tmpdir=None):
    a_full = np.ascontiguousarray(np.asarray(fxA), dtype=np.float32).reshape(
        N_SLICES, HW
    )
    b_full = np.ascontiguousarray(np.asarray(fyA), dtype=np.float32).reshape(
        N_SLICES, HW
    )
    v_full = np.ascontiguousarray(np.asarray(fyB), dtype=np.float32).reshape(
        N_SLICES, HW
    )

    in_maps = []
    for c in range(N_CORES):
        lo, hi = c * S, (c + 1) * S
        in_maps.append(
            {"a": a_full[lo:hi], "b": b_full[lo:hi], "v": v_full[lo:hi]}
        )

    res = run_bass_kernel_spmd(
        _get_nc(),
        in_maps,
        core_ids=list(range(N_CORES)),
        trace=_trace,
        tmpdir=_tmpdir,
    )
    out = np.concatenate([r["x"] for r in res.results], axis=0)
    if _trace:
        kernel.last_results = res
    return out.reshape(OUT_SHAPE).astype(np.float32)



# revision 49
# speedup vs baseline: 1.2291x; 1.0516x over previous
"""Trainium2 Bass kernel for quadratic non-softmax attention:

    X[n,c,i] = sum_j exp(a[n,c,i] * b[n,c,j]) * v[n,c,j]

with a=fxA, b=fyA, v=fyB, each (2,16,32,32) fp32 -> 32 independent
(n,c) slices of an HWxHW=1024x1024 problem. Sharded 4 slices/core
across 8 NeuronCores (pure data parallel).

Two implementations:

v1 (direct): per i-tile, ScalarE computes E[p,j]=Exp(a[i_p]*b[j]) with a
per-partition `scale`, VectorE's fused affine_mul_reduce does
sum_j E[p,j]*v[j]. ~32 ScalarE exp passes of 1024 elements per core.

v2 (grid-factored, default): quantize a_i = g_k + r_i on a 64-point grid
g_k=(k-32)*h, h=5/32 (exact in fp32, so r is computed exactly). Then
  exp(a_i b_j) = exp(g_k b_j) * exp(r_i b_j)
  X[i] = sum_d r_i^d * H_d[k_i],  H_d[k] = sum_j e^{g_k b_j} v_j b_j^d/d!
(degree-7 Taylor in r*b, |r*b| <= (h/2)*max|b| ~ 0.37 -> trunc err ~8e-9
relative). ScalarE only computes the 128x64 table exp(b_j*g_k) per
j-tile (16x less exp work); H_d comes from one accumulated fp32 matmul
per j-tile; the per-i gather H[k_i,:] is a one-hot fp32 matmul; the
Taylor evaluation is an elementwise multiply with precomputed r^d powers
plus a free-axis reduce_sum on VectorE.
"""

import os

import numpy as np

import concourse.bass as bass
import concourse.mybir as mybir
import concourse.tile as tile
from concourse import bacc
from concourse.bass_utils import run_bass_kernel_spmd

N_CORES = 8
N_SLICES = 32          # N*C = 2*16
S = N_SLICES // N_CORES  # 4 slices per core
HW = 1024              # H*W = 32*32
P = 128                # partitions
NT = HW // P           # 8 tiles of 128
OUT_SHAPE = (2, 16, 32, 32)
F32 = mybir.dt.float32
I32 = mybir.dt.int32

GRID_K = 64            # grid points (= H matmul output partitions)
GRID_H = 5.0 / 32.0    # grid step; exact in fp32 so r = a - k*h is exact
GRID_LO = -5.0         # grid value of index 0; covers |a| <= 5 + h/2
D = 8                  # Taylor terms d=0..7; |r*b| <= (h/2)*4.7 ~ 0.37,
                       # truncation |rb|^8/8! ~ 8e-9 << fp32 noise
BF16 = mybir.dt.bfloat16


def _new_nc():
    return bacc.Bacc(
        "TRN2",
        target_bir_lowering=False,
        debug=False,
        num_devices=N_CORES,
    )


def build_nc_v1() -> bass.Bass:
    nc = _new_nc()
    a = nc.dram_tensor("a", [S, HW], F32, kind="ExternalInput")
    b = nc.dram_tensor("b", [S, HW], F32, kind="ExternalInput")
    v = nc.dram_tensor("v", [S, HW], F32, kind="ExternalInput")
    x = nc.dram_tensor("x", [S, HW], F32, kind="ExternalOutput")

    with tile.TileContext(nc) as tc:
        with (
            tc.tile_pool(name="bcast", bufs=2) as bcast,
            tc.tile_pool(name="small", bufs=2) as small,
            tc.tile_pool(name="work", bufs=4) as work,
        ):
            for s in range(S):
                acol = small.tile([P, NT], F32, tag="acol")
                nc.sync.dma_start(
                    out=acol, in_=a[s].rearrange("(p t) -> p t", p=P)
                )
                bb = bcast.tile([P, HW], F32, tag="bb")
                nc.sync.dma_start(
                    out=bb, in_=b[s : s + 1, :].to_broadcast((P, HW))
                )
                vb = bcast.tile([P, HW], F32, tag="vb")
                nc.sync.dma_start(
                    out=vb, in_=v[s : s + 1, :].to_broadcast((P, HW))
                )
                xcol = small.tile([P, NT], F32, tag="xcol")
                for t in range(NT):
                    e = work.tile([P, HW], F32, tag="e")
                    nc.scalar.activation(
                        out=e,
                        in_=bb,
                        func=mybir.ActivationFunctionType.Exp,
                        scale=acol[:, t : t + 1],
                    )
                    # prod = e * vb ; xcol[:, t] = sum_j prod[:, j]
                    prod = work.tile([P, HW], F32, tag="prod")
                    nc.vector.affine_mul_reduce(
                        out=prod,
                        accum_out=xcol[:, t : t + 1],
                        in0=e,
                        in1=vb,
                        scale=1.0,
                        bias=0.0,
                    )
                nc.sync.dma_start(
                    out=x[s].rearrange("(p t) -> p t", p=P), in_=xcol
                )
    nc.compile()
    return nc


def build_nc_v2() -> bass.Bass:
    nc = _new_nc()
    a = nc.dram_tensor("a", [S, HW], F32, kind="ExternalInput")
    b = nc.dram_tensor("b", [S, HW], F32, kind="ExternalInput")
    v = nc.dram_tensor("v", [S, HW], F32, kind="ExternalInput")
    x = nc.dram_tensor("x", [S, HW], F32, kind="ExternalOutput")

    # index mapping: i = p*NT + t (and j = p*NT + u) so the DRAM side of
    # every strided DMA moves NT contiguous elements per (partition, slice)
    col_view = lambda t: t.rearrange("s (p t) -> p s t", p=P)

    with tile.TileContext(nc) as tc:
        with (
            tc.tile_pool(name="const", bufs=1) as const,
            tc.tile_pool(name="cols", bufs=1) as cols,
            tc.tile_pool(name="gt", bufs=8) as gtp,
            tc.tile_pool(name="oh", bufs=1) as ohp,
            tc.tile_pool(name="hsb", bufs=2) as hsbp,
            tc.tile_pool(name="hps", bufs=1, space="PSUM") as hps,
            tc.tile_pool(name="coefps", bufs=1, space="PSUM") as coefps,
            tc.tile_pool(name="dram", bufs=1, space="DRAM") as dram,
        ):
            # ---- constants ----
            io32 = const.tile([P, GRID_K], I32, tag="io32")
            nc.gpsimd.iota(
                out=io32, pattern=[[1, GRID_K]], base=0, channel_multiplier=0
            )
            iof = const.tile([P, GRID_K], F32, tag="iof")
            nc.vector.tensor_copy(out=iof, in_=io32)
            # gridb[p, k] = k*h + GRID_LO, same on every partition (exact fp32)
            gridb = const.tile([P, GRID_K], F32, tag="gridb")
            nc.scalar.activation(
                out=gridb,
                in_=iof,
                func=mybir.ActivationFunctionType.Copy,
                scale=GRID_H,
                bias=GRID_LO,
            )
            pc32 = const.tile([GRID_K, 1], I32, tag="pc32")
            nc.gpsimd.iota(
                out=pc32,
                pattern=[[0, 1]],
                base=int(GRID_LO / GRID_H),
                channel_multiplier=1,
            )
            pcf = const.tile([GRID_K, 1], F32, tag="pcf")
            nc.vector.tensor_copy(out=pcf, in_=pc32)

            # ---- batched input loads (column layout: [p, s, t]) ----
            acol = cols.tile([P, S, NT], F32, tag="acol")
            nc.sync.dma_start(out=acol, in_=col_view(a))
            bcol = cols.tile([P, S, NT], F32, tag="bcol")
            nc.sync.dma_start(out=bcol, in_=col_view(b))
            vcol = cols.tile([P, S, NT], F32, tag="vcol")
            nc.sync.dma_start(out=vcol, in_=col_view(v))

            # ---- grid index + remainder (column layout, all slices) ----
            # a*(1/h) with int32 output dtype = round-to-nearest in one op
            ki32 = cols.tile([P, S, NT], I32, tag="ki32")
            nc.vector.tensor_scalar_mul(out=ki32, in0=acol, scalar1=1.0 / GRID_H)
            kf = cols.tile([P, S, NT], BF16, tag="kf")
            nc.vector.tensor_copy(out=kf, in_=ki32)
            # r = a - kf*h  (exact: kf*h is exact for h=5/32, |kf|<=32)
            rc = cols.tile([P, S, NT], F32, tag="rc")
            nc.vector.scalar_tensor_tensor(
                out=rc, in0=kf, scalar=-GRID_H, in1=acol,
                op0=mybir.AluOpType.mult, op1=mybir.AluOpType.add,
            )

            # roundtrip through DRAM to get row layout for the one-hot
            # compare; kf holds k' = k-32 (exact small ints in bf16), so the
            # partition-index constant below is built with base -32
            kinl = dram.tile([S, HW], BF16, tag="kinl")
            nc.sync.dma_start(out=col_view(kinl), in_=kf)

            # ---- Taylor weights W[p, s, t, d] = v * b^d / d! ----
            W = cols.tile([P, S, NT, D], F32, tag="W")
            nc.vector.tensor_copy(out=W[:, :, :, 0], in_=vcol)
            for d in range(1, D):
                # W_d = (W_{d-1} * (1/d)) * b
                nc.vector.scalar_tensor_tensor(
                    out=W[:, :, :, d],
                    in0=W[:, :, :, d - 1],
                    scalar=1.0 / d,
                    in1=bcol,
                    op0=mybir.AluOpType.mult,
                    op1=mybir.AluOpType.mult,
                )

            # ---- powers RD[p, s, t, d] = r^d (early; off the critical path) ----
            RD = cols.tile([P, S, NT, D], F32, tag="RD")
            nc.vector.scalar_tensor_tensor(
                out=rc, in0=kf, scalar=-H3, in1=acol,
                op0=mybir.AluOpType.mult, op1=mybir.AluOpType.add,
            )
            r2 = cols.tile([P, S, NT], F32, tag="r2")
            r4 = cols.tile([P, S, NT], F32, tag="r4")
            nc.vector.memset(RD[:, :, :, 0], 1.0)
            nc.vector.tensor_copy(out=RD[:, :, :, 1], in_=rc)
            nc.vector.tensor_mul(out=r2, in0=rc, in1=rc)
            nc.vector.tensor_tensor(
                out=RD[:, :, :, 2:4], in0=RD[:, :, :, 0:2],
                in1=bcast_d(r2, 2), op=mybir.AluOpType.mult,
            )
            nc.vector.tensor_mul(out=r4, in0=r2, in1=r2)
            nc.vector.tensor_tensor(
                out=RD[:, :, :, 4:8], in0=RD[:, :, :, 0:4],
                in1=bcast_d(r4, 4), op=mybir.AluOpType.mult,
            )

            coef = coefps.tile([P, S, NT, D], F32, tag="coef")

            # ---- one-hot rows for every slice, computed up front so the
            # gather matmuls never wait on the kib broadcast DMAs ----
            ohs = []
            for pair in range(S // 2):
                # one broadcast DMA per slice-pair (halves the per-DMA fixed
                # cost paid on this chain); is_equal stays per slice
                kib2 = ohp.tile([GRID_K, 2, HW], BF16, tag=f"kib{pair}")
                kv = kinl[2 * pair : 2 * pair + 2, :]
                nc.sync.dma_start(
                    out=kib2,
                    in_=bass.AP(
                        kv.tensor,
                        kv.offset,
                        [[0, GRID_K]] + [list(d) for d in kv.ap],
                    ),
                )
                for j in range(2):
                    oh = ohp.tile([GRID_K, HW], F32, tag=f"oh{2 * pair + j}")
                    nc.vector.tensor_scalar(
                        out=oh,
                        in0=kib2[:, j, :],
                        scalar1=pcf,
                        scalar2=None,
                        op0=mybir.AluOpType.is_equal,
                    )
                    ohs.append(oh)

            for s in range(S):
                # ---- H_d[k] = sum_j e^{g_k b_j} v_j b_j^d/d!  (PSUM accum) ----
                # arg[p, u, k] = b[p,s,u] * g_k via broadcast APs (one DVE op),
                # then ONE big exp per slice (the per-instruction ACT overhead
                # dominates small tiles, so batch all 8 j-tiles' tables)
                arg = gtp.tile([P, NT, GRID_K], F32, tag="arg")
                bs = bcol[:, s, :]
                b_b = bass.AP(
                    bs.tensor, bs.offset,
                    [list(d) for d in bs.ap] + [[0, GRID_K]],
                )
                g_b = bass.AP(
                    gridb.tensor, gridb.offset,
                    [list(gridb.ap[0]), [0, NT], list(gridb.ap[1])],
                )
                nc.vector.tensor_tensor(
                    out=arg, in0=b_b, in1=g_b, op=mybir.AluOpType.mult
                )
                gt = gtp.tile([P, NT, GRID_K], F32, tag="gt")
                nc.scalar.activation(
                    out=gt, in_=arg, func=mybir.ActivationFunctionType.Exp
                )
                Hps = hps.tile([GRID_K, D], F32, tag="H")
                for u in range(NT):
                    nc.tensor.matmul(
                        out=Hps,
                        lhsT=gt[:, u, :],
                        rhs=W[:, s, u, :],
                        start=(u == 0),
                        stop=(u == NT - 1),
                    )
                Hsb = hsbp.tile([GRID_K, D], F32, tag="Hsb")
                # PSUM->SBUF handoff on the (idle) scalar engine so it never
                # queues behind VectorE work and PSUM banks recycle promptly
                nc.scalar.copy(out=Hsb, in_=Hps)

                # ---- one-hot gather: coef[i, :] = H[k_i, :] ----
                # free position in oh is i = c*NT + u; tile u gathers the
                # i's congruent to u (strided columns), matching coef[c,s,u]
                oh_r = ohs[s].rearrange("p (c t) -> p t c", t=NT)
                for u in range(NT):
                    nc.tensor.matmul(
                        out=coef[:, s, u, :],
                        lhsT=oh_r[:, u, :],
                        rhs=Hsb,
                        start=True,
                        stop=True,
                    )

            # ---- X = sum_d r^d * H_d[k_i]: multiply + reduce, in two halves
            # so the first half's store overlaps the second half's compute ----
            half = S // 2
            for lo in (0, half):
                hi = lo + half
                prodc = cols.tile([P, half, NT, D], F32, tag=f"prodc{lo}")
                nc.vector.tensor_mul(
                    out=prodc, in0=RD[:, lo:hi, :, :], in1=coef[:, lo:hi, :, :]
                )
                xall = cols.tile([P, half, NT], F32, tag=f"xall{lo}")
                nc.vector.reduce_sum(
                    out=xall, in_=prodc, axis=mybir.AxisListType.X
                )
                nc.sync.dma_start(
                    out=x[lo:hi].rearrange("s (p t) -> p s t", p=P), in_=xall
                )
    nc.compile()
    return nc


K3 = 32                # v3 grid points; h exact in fp32
H3 = 5.0 / 16.0        # 0.3125 = 5 * 2^-4
KOFF = 16              # partition (s*K3 + k) holds grid index k' = k - KOFF
INV_H3 = 1.0 / H3      # fl(3.2); both rounding ops use this same constant


def build_nc_v3() -> bass.Bass:
    """v3: no DRAM roundtrip for the one-hot. `a` is ALSO loaded in a
    row-broadcast layout abc[s*32+k, i] = a[s, i]; GPSIMD rounds both the
    column copy (acol -> ki32, for the Taylor remainder r) and the row
    copy (abc -> kint, for the one-hot compare) with the same op on the
    same engine, so the two roundings are bit-identical by construction.
    DMAs are split across both HWDGE queues (SP + Activation). K=32
    halves the exp-table/arg work vs v2's K=64 (|r*b| <= 0.16*4.4 ->
    Taylor-8 truncation ~1e-6 relative, well below fp32 noise)."""
    nc = _new_nc()
    a = nc.dram_tensor("a", [S, HW], F32, kind="ExternalInput")
    b = nc.dram_tensor("b", [S, HW], F32, kind="ExternalInput")
    v = nc.dram_tensor("v", [S, HW], F32, kind="ExternalInput")
    x = nc.dram_tensor("x", [S, HW], F32, kind="ExternalOutput")

    col_view = lambda t: t.rearrange("s (p t) -> p s t", p=P)

    with tile.TileContext(nc) as tc:
        with (
            tc.tile_pool(name="const", bufs=1) as const,
            tc.tile_pool(name="cols", bufs=1) as cols,
            tc.tile_pool(name="big", bufs=1) as big,
            tc.tile_pool(name="gtp", bufs=2) as gtp,
            tc.tile_pool(name="hsb", bufs=2) as hsbp,
            tc.tile_pool(name="outp", bufs=2) as outp,
            tc.tile_pool(name="hps", bufs=1, space="PSUM") as hps,
            tc.tile_pool(name="coefps", bufs=1, space="PSUM") as coefps,
        ):
            # ---- input DMAs ----
            # TimelineSim serializes HWDGE slots (~625ns each), so ORDER is
            # the schedule: abc gates the longest chain (kint -> oh ->
            # gather), bcol the args -> exp -> H chain.
            nch = int(os.environ.get("V3_ABC_CHUNKS", "2"))
            order = os.environ.get("V3_DMA_ORDER", "b,g0,a,g1,v")
            QW = HW // nch
            abc = big.tile([P, HW], F32, tag="abc")
            aap = a[:, :]
            cols_tiles = {}
            engs = [nc.sync, nc.scalar]
            ei = 0

            def emit_dma(key):
                nonlocal ei
                eng = engs[ei % 2]
                ei += 1
                if key == "b":
                    t = cols.tile([P, S, NT], F32, tag="bcol")
                    eng.dma_start(out=t, in_=col_view(b))
                    cols_tiles["b"] = t
                elif key == "a":
                    t = cols.tile([P, S, NT], F32, tag="acol")
                    eng.dma_start(out=t, in_=col_view(a))
                    cols_tiles["a"] = t
                elif key == "v":
                    t = cols.tile([P, S, NT], F32, tag="vcol")
                    eng.dma_start(out=t, in_=col_view(v))
                    cols_tiles["v"] = t
                elif key.startswith("g"):
                    q = int(key[1:])
                    eng.dma_start(
                        out=abc[:, q * QW : (q + 1) * QW],
                        in_=bass.AP(
                            aap.tensor,
                            aap.offset + q * QW,
                            [[HW, S], [0, K3], [1, QW]],
                        ),
                    )

            for key in order.split(","):
                emit_dma(key)
            bcol = cols_tiles["b"]
            acol = cols_tiles["a"]
            vcol = cols_tiles["v"]

            # ---- constants ----
            io32 = const.tile([P, K3], I32, tag="io32")
            nc.gpsimd.iota(
                out=io32, pattern=[[1, K3]], base=0, channel_multiplier=0
            )
            iof = const.tile([P, K3], F32, tag="iof")
            nc.vector.tensor_copy(out=iof, in_=io32)
            # gridb[p, k] = (k - KOFF) * h, same on every partition
            gridb = const.tile([P, K3], F32, tag="gridb")
            nc.scalar.activation(
                out=gridb,
                in_=iof,
                func=mybir.ActivationFunctionType.Copy,
                scale=H3,
                bias=-KOFF * H3,
            )
            # pci[p] = (p & 31) - KOFF : the k' each oh partition matches
            piota = const.tile([P, 1], I32, tag="piota")
            nc.gpsimd.iota(
                out=piota, pattern=[[0, 1]], base=0, channel_multiplier=1
            )
            pcim = const.tile([P, 1], I32, tag="pcim")
            nc.vector.tensor_scalar(
                out=pcim,
                in0=piota,
                scalar1=K3 - 1,
                scalar2=None,
                op0=mybir.AluOpType.bitwise_and,
            )
            pcii = const.tile([P, 1], I32, tag="pcii")
            nc.vector.tensor_scalar_add(out=pcii, in0=pcim, scalar1=-KOFF)
            pci = const.tile([P, 1], F32, tag="pci")
            nc.vector.tensor_copy(out=pci, in_=pcii)
            import math as _math
            facinv = const.tile([P, D], F32, tag="facinv")
            for d in range(D):
                nc.vector.memset(facinv[:, d : d + 1], 1.0 / _math.factorial(d))

            # ---- GPSIMD rounds both layouts (bit-identical) ----
            ki32 = cols.tile([P, S, NT], I32, tag="ki32")
            nc.gpsimd.tensor_scalar_mul(out=ki32, in0=acol, scalar1=INV_H3)
            kint = big.tile([P, HW], I32, tag="kint")
            for q in range(nch):
                nc.gpsimd.tensor_scalar_mul(
                    out=kint[:, q * QW : (q + 1) * QW],
                    in0=abc[:, q * QW : (q + 1) * QW],
                    scalar1=INV_H3,
                )

            # ---- DVE: args (pairwise), remainder, powers ----
            arg = big.tile([P, S, NT, K3], F32, tag="arg")
            W = cols.tile([P, S, NT, D], F32, tag="W")
            RD = cols.tile([P, S, NT, D], F32, tag="RD")
            oh = big.tile([P, HW], F32, tag="oh")
            kf = cols.tile([P, S, NT], BF16, tag="kf")
            rc = cols.tile([P, S, NT], F32, tag="rc")

            def arg_pair(lo):
                bs = bcol[:, lo : lo + 2, :]
                b_b = bass.AP(
                    bs.tensor,
                    bs.offset,
                    [list(d) for d in bs.ap] + [[0, K3]],
                )
                g_b = bass.AP(
                    gridb.tensor,
                    gridb.offset,
                    [list(gridb.ap[0]), [0, 2], [0, NT], list(gridb.ap[1])],
                )
                nc.vector.tensor_tensor(
                    out=arg[:, lo : lo + 2], in0=b_b, in1=g_b,
                    op=mybir.AluOpType.mult,
                )

            with tc.high_priority():
                arg_pair(0)
                arg_pair(2)

            # W_d = v * b^d / d! — build b^d/d! on GPSIMD (needs only bcol;
            # keeps DVE free for args), then per-pair multiplies by v on DVE
            # unnormalized powers b^d by doubling (1/d! is applied later,
            # folded into the Hsbd copy); fewer DVE ops than a 7-step chain
            B = cols.tile([P, S, NT, D], F32, tag="B")
            b2 = cols.tile([P, S, NT], F32, tag="b2")
            b4 = cols.tile([P, S, NT], F32, tag="b4")
            nc.vector.memset(B[:, :, :, 0], 1.0)
            nc.vector.tensor_copy(out=B[:, :, :, 1], in_=bcol)
            nc.vector.tensor_mul(out=b2, in0=bcol, in1=bcol)

            def bcast_d(t, n):
                return bass.AP(
                    t.tensor, t.offset,
                    [list(dd) for dd in t.ap] + [[0, n]],
                )

            nc.vector.tensor_tensor(
                out=B[:, :, :, 2:4], in0=B[:, :, :, 0:2],
                in1=bcast_d(b2, 2), op=mybir.AluOpType.mult,
            )
            nc.vector.tensor_mul(out=b4, in0=b2, in1=b2)
            nc.vector.tensor_tensor(
                out=B[:, :, :, 4:8], in0=B[:, :, :, 0:4],
                in1=bcast_d(b4, 4), op=mybir.AluOpType.mult,
            )
            nc.gpsimd.tensor_copy(out=kf, in_=ki32)
            for lo in (0, 2):
                vs = vcol[:, lo : lo + 2, :]
                v_b = bass.AP(
                    vs.tensor, vs.offset,
                    [list(dd) for dd in vs.ap] + [[0, D]],
                )
                nc.gpsimd.tensor_tensor(
                    out=W[:, lo : lo + 2], in0=B[:, lo : lo + 2], in1=v_b,
                    op=mybir.AluOpType.mult,
                )
            # one-hot in free-chunks (pipelines behind the kint chunks)
            for q in range(nch):
                nc.vector.tensor_scalar(
                    out=oh[:, q * QW : (q + 1) * QW],
                    in0=kint[:, q * QW : (q + 1) * QW],
                    scalar1=pci,
                    scalar2=None,
                    op0=mybir.AluOpType.is_equal,
                )
            # RD_d = r^d for the final Taylor eval (fills the DVE gap
            # before the kint chunks land)
            nc.vector.scalar_tensor_tensor(
                out=rc, in0=kf, scalar=-H3, in1=acol,
                op0=mybir.AluOpType.mult, op1=mybir.AluOpType.add,
            )
            r2 = cols.tile([P, S, NT], F32, tag="r2")
            r4 = cols.tile([P, S, NT], F32, tag="r4")
            nc.vector.memset(RD[:, :, :, 0], 1.0)
            nc.vector.tensor_copy(out=RD[:, :, :, 1], in_=rc)
            nc.vector.tensor_mul(out=r2, in0=rc, in1=rc)
            nc.vector.tensor_tensor(
                out=RD[:, :, :, 2:4], in0=RD[:, :, :, 0:2],
                in1=bcast_d(r2, 2), op=mybir.AluOpType.mult,
            )
            nc.vector.tensor_mul(out=r4, in0=r2, in1=r2)
            nc.vector.tensor_tensor(
                out=RD[:, :, :, 4:8], in0=RD[:, :, :, 0:4],
                in1=bcast_d(r4, 4), op=mybir.AluOpType.mult,
            )

            # ---- ACT: exp tables (pairwise); PE: H matmuls ----
            # Hall[32s + k, d] = H_d^{(s)}[k]: slice s's table lives at
            # partition base 32s (PE tile_position writes PSUM there), so
            # the gather's lhsT (oh rows) and rhs (Hsbd rows) share a base.
            # Hsbd is BLOCK-DIAGONAL per pair ([H_2p | 0; 0 | H_2p+1]) so
            # one matmul gathers both slices of a pair: 16 gathers, not 32.
            coefd0 = coefps.tile([P, NT, 2, D], F32, tag="coefd0")
            coefd1 = coefps.tile([P, NT, 2, D], F32, tag="coefd1")
            coefd = [coefd0, coefd1]
            Hall0 = hps.tile([P, D], F32, tag="Hall0")
            Hall1 = hps.tile([P, D], F32, tag="Hall1")
            Hall2 = hps.tile([P, D], F32, tag="Hall2")
            Hall3 = hps.tile([P, D], F32, tag="Hall3")
            Halls = [Hall0, Hall1, Hall2, Hall3]
            Hsbd = hsbp.tile([P, 2, D], F32, tag="Hsbd")
            nc.vector.memset(Hsbd, 0.0)
            for pair in range(2):
                lo = 2 * pair
                gt = gtp.tile([P, 2, NT, K3], F32, tag=f"gt{pair}")
                nc.scalar.activation(
                    out=gt, in_=arg[:, lo : lo + 2],
                    func=mybir.ActivationFunctionType.Exp,
                )
                for j in range(2):
                    s = lo + j
                    for u in range(NT):
                        nc.tensor.matmul(
                            out=Halls[s][K3 * s : K3 * s + K3, :],
                            lhsT=gt[:, j, u, :],
                            rhs=W[:, s, u, :],
                            start=(u == 0),
                            stop=(u == NT - 1),
                            tile_position=(0, K3 * s),
                        )
                    nc.vector.tensor_mul(
                        out=Hsbd[K3 * s : K3 * s + K3, j, :],
                        in0=Halls[s][K3 * s : K3 * s + K3, :],
                        in1=facinv[K3 * s : K3 * s + K3, :],
                    )
                # merged gather: coefd[c, pair, u, j, :] = H_{2p+j}[k'_..., :]
                oh_r = oh[64 * pair : 64 * pair + 64].rearrange(
                    "k (c t) -> k t c", t=NT
                )
                for u in range(NT):
                    nc.tensor.matmul(
                        out=coefd[pair][:, u, :, :],
                        lhsT=oh_r[:, u, :],
                        rhs=Hsbd[64 * pair : 64 * pair + 64, :, :],
                        start=True,
                        stop=True,
                        tile_position=(64 * pair, 0),
                    )

            # ---- final eval + stores, pairwise so store0 overlaps pair 1 ----
            prodc = outp.tile([P, S, NT, D], F32, tag="prodc")
            xall = outp.tile([P, S, NT], F32, tag="xall")
            for pair, eng in ((0, nc.sync), (1, nc.scalar)):
                lo = 2 * pair
                nc.vector.tensor_mul(
                    out=prodc[:, lo : lo + 2],
                    in0=RD[:, lo : lo + 2],
                    in1=coefd[pair].rearrange("p t j d -> p j t d"),
                )
                nc.vector.reduce_sum(
                    out=xall[:, lo : lo + 2],
                    in_=prodc[:, lo : lo + 2],
                    axis=mybir.AxisListType.X,
                )
                eng.dma_start(
                    out=x[lo : lo + 2].rearrange("s (p t) -> p s t", p=P),
                    in_=xall[:, lo : lo + 2],
                )
    nc.compile()
    return nc


_NC_CACHE = {}
_VERSION = os.environ.get("KERNEL_VERSION", "v3")


def _get_nc():
    ver = _VERSION
    if ver not in _NC_CACHE:
        builders = {"v1": build_nc_v1, "v2": build_nc_v2, "v3": build_nc_v3}
        _NC_CACHE[ver] = builders[ver]()
    return _NC_CACHE[ver]


def kernel(fxA, fyA, fyB, _trace=False, _tmpdir=None):
    a_full = np.ascontiguousarray(np.asarray(fxA), dtype=np.float32).reshape(
        N_SLICES, HW
    )
    b_full = np.ascontiguousarray(np.asarray(fyA), dtype=np.float32).reshape(
        N_SLICES, HW
    )
    v_full = np.ascontiguousarray(np.asarray(fyB), dtype=np.float32).reshape(
        N_SLICES, HW
    )

    in_maps = []
    for c in range(N_CORES):
        lo, hi = c * S, (c + 1) * S
        in_maps.append(
            {"a": a_full[lo:hi], "b": b_full[lo:hi], "v": v_full[lo:hi]}
        )

    res = run_bass_kernel_spmd(
        _get_nc(),
        in_maps,
        core_ids=list(range(N_CORES)),
        trace=_trace,
        tmpdir=_tmpdir,
    )
    out = np.concatenate([r["x"] for r in res.results], axis=0)
    if _trace:
        kernel.last_results = res
    return out.reshape(OUT_SHAPE).astype(np.float32)

